# revision 14
# baseline (speedup 1.0000x reference)
"""AttnDecoderRNN single-step kernel for 8 Trainium2 NeuronCores.

Full (unsharded) inputs in, full outputs out.  Internally:
  - attention: L=2048 encoder rows sharded 256/core; softmax + context
    combined with a single AllGather of (local max, local sumexp, partial
    context) per core.
  - LSTM: gate outputs column-sharded 128/core.
  - output layer: V=50257 column-sharded 6283/core (V padded to 50264);
    the K=4096 contraction is split into [emb|hidden] rows (ready at t=0)
    and [context] rows (ready after the AllGather) so the big W_out DMA
    stream never stalls.
  - log-softmax: local max/sumexp per V-shard + a second tiny AllGather.
All math is fp32 on device; the host only routes/reshapes data.
"""

import os
import sys

import numpy as np

if "/opt/trn_rl_repo" not in sys.path:
    sys.path.insert(0, "/opt/trn_rl_repo")

import concourse.bacc as bacc
import concourse.bass as bass
import concourse.mybir as mybir
import concourse.tile as tile
from concourse.bass_utils import run_bass_kernel_spmd

NCORES = 8
H = 1024
V = 50257
L = 2048
LS = L // NCORES            # 256 encoder rows per core
VS = 6283                   # padded vocab shard (8*6283 = 50264 >= V)
NEG_BIG = -1.0e30

F32 = mybir.dt.float32
AF = mybir.ActivationFunctionType
ALU = mybir.AluOpType
AX = mybir.AxisListType
RG = [list(range(NCORES))]


def _chunks(n, step=512):
    out = []
    c0 = 0
    while c0 < n:
        out.append((c0, min(step, n - c0)))
        c0 += step
    return out


def build_program(vs=VS):
    """Build + compile the SPMD Bass program (identical on all 8 cores)."""
    from contextlib import ExitStack

    nc = bacc.Bacc(
        "TRN2",
        target_bir_lowering=False,
        debug=False,
        num_devices=NCORES,
    )

    # ---- I/O ----
    enc_d = nc.dram_tensor("enc", [LS, 2 * H], F32, kind="ExternalInput")
    wattn_d = nc.dram_tensor("w_attn", [3 * H, H], F32, kind="ExternalInput")
    wout_d = nc.dram_tensor("w_out", [4 * H, vs], F32, kind="ExternalInput")
    bout_d = nc.dram_tensor("b_out", [1, vs], F32, kind="ExternalInput")
    wg_d = nc.dram_tensor("w_g", [4, 4 * H, 128], F32, kind="ExternalInput")
    vecs_d = nc.dram_tensor("vecs", [128, 32], F32, kind="ExternalInput")
    rows_d = nc.dram_tensor("rows", [1, 1664], F32, kind="ExternalInput")
    ident_d = nc.dram_tensor("ident", [128, 128], F32, kind="ExternalInput")

    o_lsm = nc.dram_tensor("o_lsm", [1, vs], F32, kind="ExternalOutput")
    o_h = nc.dram_tensor("o_h", [1, 128], F32, kind="ExternalOutput")
    o_c = nc.dram_tensor("o_c", [1, 128], F32, kind="ExternalOutput")
    o_aw = nc.dram_tensor("o_aw", [1, LS], F32, kind="ExternalOutput")

    chs = _chunks(vs)
    npart = 512 * len(chs)  # partial row width (512-aligned slots per chunk)

    with tile.TileContext(nc) as tc, ExitStack() as ctx:
        pc = ctx.enter_context(tc.tile_pool(name="const", bufs=1))
        pw_out = ctx.enter_context(tc.tile_pool(name="pwout", bufs=2))
        pw_attn = ctx.enter_context(tc.tile_pool(name="pwattn", bufs=2))
        pw_misc = ctx.enter_context(tc.tile_pool(name="pwmisc", bufs=1))
        ps_a = ctx.enter_context(tc.tile_pool(name="psA", bufs=2, space="PSUM"))
        ps_t = ctx.enter_context(tc.tile_pool(name="psT", bufs=2, space="PSUM"))
        ps_r = ctx.enter_context(tc.tile_pool(name="psR", bufs=3, space="PSUM"))
        pdram = ctx.enter_context(tc.tile_pool(name="dram", bufs=1, space="DRAM"))

        # ---- constants / small vectors ----
        vecs_sb = pc.tile([128, 32], F32, name="vecs_sb")
        nc.scalar.dma_start(vecs_sb[:], vecs_d[:, :])
        ident_sb = pc.tile([128, 128], F32, name="ident_sb")
        nc.scalar.dma_start(ident_sb[:], ident_d[:, :])
        rows_sb = pc.tile([1, 1664], F32, name="rows_sb")
        nc.scalar.dma_start(rows_sb[:], rows_d[:, :])
        ones_sb = pc.tile([1, 1], F32, name="ones_sb")
        nc.vector.memset(ones_sb[:], 1.0)

        # emb (+ b_emb) in transposed column layout [128, 8]
        emb_f = pc.tile([128, 8], F32, name="emb_f")
        nc.vector.tensor_add(emb_f[:], vecs_sb[:, 0:8], vecs_sb[:, 8:16])
        hidT = vecs_sb[:, 16:24]     # hidden.T columns
        waoT = vecs_sb[:, 24:32]     # W_attn_out as 8 lhsT columns [128,1]

        # ---- encoder rows: natural + transposed ----
        # enc_sb[:, t*2H + c] = enc[t*128 + p, c]   (t = 0,1)
        enc_sb = pc.tile([128, 2 * 2 * H], F32, name="enc_sb")
        nc.scalar.dma_start(
            enc_sb[:], enc_d.ap().rearrange("(t p) c -> p t c", p=128)
        )
        # encT[:, k*LS + t*128 + l] = enc[t*128 + l, k*128 + p]
        encT = pc.tile([128, 16 * LS], F32, name="encT")
        for t in range(2):
            for k in range(16):
                ps = ps_t.tile([128, 128], F32, name="tp_ps", tag="psT")
                nc.tensor.transpose(
                    ps[:],
                    enc_sb[:, t * 2 * H + k * 128 : t * 2 * H + (k + 1) * 128],
                    ident_sb[:],
                )
                nc.vector.tensor_copy(
                    encT[:, k * LS + t * 128 : k * LS + (t + 1) * 128], ps[:]
                )

        # ---- h_row = hidden @ W_attn[2H:3H, :] + b_attn  -> [1, 1024] ----
        hrow_sb = pc.tile([1, H], F32, name="hrow_sb")
        for n in range(4):
            wh = pw_misc.tile([128, 8 * 256], F32, name="wh", tag="wmisc")
            nc.scalar.dma_start(
                wh[:],
                wattn_d[2 * H : 3 * H, n * 256 : (n + 1) * 256]
                .rearrange("(k p) c -> p k c", p=128),
            )
            psh = ps_r.tile([1, 512], F32, name="psh", tag="psR")
            for k in range(8):
                nc.tensor.matmul(
                    psh[0:1, 0:256],
                    hidT[:, k : k + 1],
                    wh[:, k * 256 : (k + 1) * 256],
                    start=(k == 0),
                    stop=False,
                )
            # += b_attn chunk (rank-1 trick: ones^T @ b_row)
            nc.tensor.matmul(
                psh[0:1, 0:256],
                ones_sb[:],
                rows_sb[0:1, n * 256 : (n + 1) * 256],
                start=False,
                stop=True,
            )
            nc.vector.tensor_copy(
                hrow_sb[0:1, n * 256 : (n + 1) * 256], psh[0:1, 0:256]
            )

        # transpose h_row -> per-feature bias columns biasT [128, 8]
        biasT = pc.tile([128, 8], F32, name="biasT")
        for m in range(8):
            ps = ps_t.tile([128, 1], F32, name="r2c_ps", tag="psT")
            nc.tensor.matmul(
                ps[:],
                hrow_sb[0:1, m * 128 : (m + 1) * 128],
                ones_sb[:],
                start=True,
                stop=True,
            )
            nc.vector.tensor_copy(biasT[:, m : m + 1], ps[:])

        # ---- scores: tanh(enc @ W1 + h_row + b_attn) @ W_attn_out ----
        score_ps = ps_r.tile([1, LS], F32, name="score_ps", tag="psR")
        for m in range(8):
            wa = pw_attn.tile([128, 16 * 128], F32, name="wa", tag="wattn")
            nc.scalar.dma_start(
                wa[:],
                wattn_d[0 : 2 * H, m * 128 : (m + 1) * 128]
                .rearrange("(k p) c -> p k c", p=128),
            )
            pe = ps_a.tile([128, LS], F32, name="pe", tag="psA")
            for k in range(16):
                nc.tensor.matmul(
                    pe[:],
                    wa[:, k * 128 : (k + 1) * 128],
                    encT[:, k * LS : (k + 1) * LS],
                    start=(k == 0),
                    stop=(k == 15),
                )
            th = pc.tile([128, LS], F32, name="th", tag="tanhT", bufs=2)
            nc.scalar.activation(th[:], pe[:], AF.Tanh, bias=biasT[:, m : m + 1])
            nc.tensor.matmul(
                score_ps[:],
                waoT[:, m : m + 1],
                th[:],
                start=(m == 0),
                stop=(m == 7),
            )

        # ---- local softmax stats + partial context ----
        m_l = pc.tile([1, 1], F32, name="m_l")
        nc.vector.reduce_max(m_l[:], score_ps[:], axis=AX.X)
        neg_ml = pc.tile([1, 1], F32, name="neg_ml")
        nc.vector.reduce_max(neg_ml[:], score_ps[:], axis=AX.X, negate=True)
        e_row = pc.tile([1, LS], F32, name="e_row")
        S_l = pc.tile([1, 1], F32, name="S_l")
        nc.scalar.activation(
            e_row[:], score_ps[:], AF.Exp, bias=neg_ml[:], accum_out=S_l[:]
        )
        # e as columns for the context matmul
        ecol = pc.tile([128, 2], F32, name="ecol")
        for j in range(2):
            ps = ps_t.tile([128, 1], F32, name="ec_ps", tag="psT")
            nc.tensor.matmul(
                ps[:],
                e_row[0:1, j * 128 : (j + 1) * 128],
                ones_sb[:],
                start=True,
                stop=True,
            )
            nc.vector.tensor_copy(ecol[:, j : j + 1], ps[:])

        stats = pc.tile([1, 2056], F32, name="stats")
        nc.vector.tensor_copy(stats[0:1, 0:1], m_l[:])
        nc.vector.tensor_copy(stats[0:1, 1:2], S_l[:])
        nc.vector.memset(stats[0:1, 2:8], 0.0)
        for n in range(4):
            ps = ps_r.tile([1, 512], F32, name="ctxp_ps", tag="psR")
            for j in range(2):
                nc.tensor.matmul(
                    ps[:],
                    ecol[:, j : j + 1],
                    enc_sb[:, j * 2 * H + n * 512 : j * 2 * H + (n + 1) * 512],
                    start=(j == 0),
                    stop=(j == 1),
                )
            nc.vector.tensor_copy(stats[0:1, 8 + n * 512 : 8 + (n + 1) * 512], ps[:])

        # ---- AllGather #1: (max, sumexp, partial context) ----
        cc1_in = pdram.tile([1, 2056], F32, name="cc1_in")
        cc1_out = pdram.tile([8, 2056], F32, name="cc1_out", addr_space="Shared")
        nc.scalar.dma_start(cc1_in[:], stats[:])
        nc.gpsimd.collective_compute(
            "AllGather",
            ALU.bypass,
            replica_groups=RG,
            ins=[cc1_in.opt()],
            outs=[cc1_out.opt()],
        )

        crows = pc.tile([8, 2 * H], F32, name="crows")
        nc.scalar.dma_start(crows[:], cc1_out[0:8, 8:2056])
        ms1 = pc.tile([1, 8], F32, name="ms1")
        nc.scalar.dma_start(ms1[:], cc1_out.rearrange("r c -> c r")[0:1, 0:8])
        ss1 = pc.tile([1, 8], F32, name="ss1")
        nc.scalar.dma_start(ss1[:], cc1_out.rearrange("r c -> c r")[1:2, 0:8])

        # global combine (all on partition 0)
        negMg = pc.tile([1, 1], F32, name="negMg")
        nc.vector.reduce_max(negMg[:], ms1[:], axis=AX.X, negate=True)
        sc1 = pc.tile([1, 8], F32, name="sc1")
        nc.scalar.activation(sc1[:], ms1[:], AF.Exp, bias=negMg[:])
        t1 = pc.tile([1, 8], F32, name="t1")
        nc.vector.tensor_mul(t1[:], sc1[:], ss1[:])
        S_tot = pc.tile([1, 1], F32, name="S_tot")
        nc.vector.reduce_sum(S_tot[:], t1[:], axis=AX.X)
        rinv = pc.tile([1, 1], F32, name="rinv")
        nc.vector.reciprocal(rinv[:], S_tot[:])
        sc2 = pc.tile([1, 8], F32, name="sc2")
        nc.vector.tensor_scalar_mul(sc2[:], sc1[:], rinv[:])

        # attn_weights output for this core's L-shard
        aw_s = pc.tile([1, 1], F32, name="aw_s")
        nc.scalar.activation(aw_s[:], m_l[:], AF.Exp, bias=negMg[:])
        aw_s2 = pc.tile([1, 1], F32, name="aw_s2")
        nc.vector.tensor_mul(aw_s2[:], aw_s[:], rinv[:])
        aw_row = pc.tile([1, LS], F32, name="aw_row")
        nc.vector.tensor_scalar_mul(aw_row[:], e_row[:], aw_s2[:])
        nc.scalar.dma_start(o_aw[:, :], aw_row[:])

        # context = sc2 @ crows  -> [1, 2048]
        sc2c_ps = ps_t.tile([8, 1], F32, name="sc2c_ps", tag="psT")
        nc.tensor.matmul(sc2c_ps[:], sc2[:], ones_sb[:], start=True, stop=True)
        sc2col = pc.tile([8, 1], F32, name="sc2col")
        nc.vector.tensor_copy(sc2col[:], sc2c_ps[:])
        ctx_row = pc.tile([1, 2 * H], F32, name="ctx_row")
        for n in range(4):
            ps = ps_r.tile([1, 512], F32, name="ctx_ps", tag="psR")
            nc.tensor.matmul(
                ps[:],
                sc2col[:],
                crows[:, n * 512 : (n + 1) * 512],
                start=True,
                stop=True,
            )
            nc.vector.tensor_copy(ctx_row[0:1, n * 512 : (n + 1) * 512], ps[:])

        # context transposed columns ctxT [128, 16]
        ctxT = pc.tile([128, 16], F32, name="ctxT")
        for t in range(16):
            ps = ps_t.tile([128, 1], F32, name="ctxT_ps", tag="psT")
            nc.tensor.matmul(
                ps[:],
                ctx_row[0:1, t * 128 : (t + 1) * 128],
                ones_sb[:],
                start=True,
                stop=True,
            )
            nc.vector.tensor_copy(ctxT[:, t : t + 1], ps[:])

        def xT_lstm(k):
            # LSTM input order: [emb, context, hidden]
            if k < 8:
                return emb_f[:, k : k + 1]
            if k < 24:
                return ctxT[:, k - 8 : k - 7]
            return hidT[:, k - 24 : k - 23]

        # ---- LSTM gates (this core's 128 output columns) ----
        gates = []
        for g in range(4):
            psg = ps_r.tile([1, 128], F32, name="psg", tag="psR")
            for half in range(2):
                wg = pw_misc.tile([128, 16 * 128], F32, name="wg", tag="wmisc")
                nc.scalar.dma_start(
                    wg[:],
                    wg_d[g][half * 2 * H : (half + 1) * 2 * H, :]
                    .rearrange("(k p) c -> p k c", p=128),
                )
                for kk in range(16):
                    k = half * 16 + kk
                    nc.tensor.matmul(
                        psg[:],
                        xT_lstm(k),
                        wg[:, kk * 128 : (kk + 1) * 128],
                        start=(k == 0),
                        stop=(k == 31),
                    )
            gsum = pc.tile([1, 128], F32, name=f"gsum{g}", tag=f"gsum{g}")
            nc.vector.tensor_add(
                gsum[:], psg[:], rows_sb[0:1, 1024 + g * 128 : 1024 + (g + 1) * 128]
            )
            gact = pc.tile([1, 128], F32, name=f"gact{g}", tag=f"gact{g}")
            nc.scalar.activation(
                gact[:], gsum[:], AF.Tanh if g == 3 else AF.Sigmoid
            )
            gates.append(gact)
        i_g, o_g, f_g, c_g = gates

        fc = pc.tile([1, 128], F32, name="fc")
        nc.vector.tensor_mul(fc[:], f_g[:], rows_sb[0:1, 1536:1664])
        ic = pc.tile([1, 128], F32, name="ic")
        nc.vector.tensor_mul(ic[:], i_g[:], c_g[:])
        cnew = pc.tile([1, 128], F32, name="cnew")
        nc.vector.tensor_add(cnew[:], fc[:], ic[:])
        nc.scalar.dma_start(o_c[:, :], cnew[:])
        ctanh = pc.tile([1, 128], F32, name="ctanh")
        nc.scalar.activation(ctanh[:], cnew[:], AF.Tanh)
        hnew = pc.tile([1, 128], F32, name="hnew")
        nc.vector.tensor_mul(hnew[:], o_g[:], ctanh[:])
        nc.scalar.dma_start(o_h[:, :], hnew[:])

        # ---- output layer: out = [emb|hidden|ctx] @ W_out + b_out ----
        # partial holds phase-1 sums per 512-slot, then (slots being
        # contiguous for every chunk but the ragged last) the final out row.
        partial = pc.tile([1, npart], F32, name="partial")

        # phase 1: K rows 0:2048 ([emb | hidden]) — no ctx dependency
        for ci, (c0, cn) in enumerate(chs):
            w1 = pw_out.tile([128, 16 * 512], F32, name="w1", tag="wout")
            nc.sync.dma_start(
                w1[:, 0 : 16 * cn],
                wout_d[0 : 2 * H, c0 : c0 + cn].rearrange(
                    "(k p) c -> p k c", p=128
                ),
            )
            ps = ps_r.tile([1, 512], F32, name="po1", tag="psR")
            for k in range(16):
                lhs = emb_f[:, k : k + 1] if k < 8 else hidT[:, k - 8 : k - 7]
                nc.tensor.matmul(
                    ps[0:1, 0:cn],
                    lhs,
                    w1[:, k * cn : (k + 1) * cn],
                    start=(k == 0),
                    stop=(k == 15),
                )
            nc.vector.tensor_copy(
                partial[0:1, ci * 512 : ci * 512 + cn], ps[0:1, 0:cn]
            )

        # phase 2: K rows 2048:4096 (context) + finalize
        for ci, (c0, cn) in enumerate(chs):
            w2 = pw_out.tile([128, 16 * 512], F32, name="w2", tag="wout")
            nc.sync.dma_start(
                w2[:, 0 : 16 * cn],
                wout_d[2 * H : 4 * H, c0 : c0 + cn].rearrange(
                    "(k p) c -> p k c", p=128
                ),
            )
            ps = ps_r.tile([1, 512], F32, name="po2", tag="psR")
            for k in range(16):
                nc.tensor.matmul(
                    ps[0:1, 0:cn],
                    ctxT[:, k : k + 1],
                    w2[:, k * cn : (k + 1) * cn],
                    start=(k == 0),
                    stop=(k == 15),
                )
            bout_c = pc.tile([1, 512], F32, name="bout_c", tag="boutc", bufs=2)
            nc.scalar.dma_start(bout_c[0:1, 0:cn], bout_d[0:1, c0 : c0 + cn])
            nc.vector.tensor_add(
                ps[0:1, 0:cn], ps[0:1, 0:cn], partial[0:1, ci * 512 : ci * 512 + cn]
            )
            nc.vector.tensor_add(
                partial[0:1, c0 : c0 + cn], ps[0:1, 0:cn], bout_c[0:1, 0:cn]
            )

        # ---- log-softmax (chunked local max / sum-exp over the out row) ----
        mx = pc.tile([1, 16], F32, name="mx")
        sx = pc.tile([1, 16], F32, name="sx")
        for ci, (c0, cn) in enumerate(chs):
            nc.vector.reduce_max(
                mx[0:1, ci : ci + 1], partial[0:1, c0 : c0 + cn], axis=AX.X
            )
        m2 = pc.tile([1, 1], F32, name="m2")
        nc.vector.reduce_max(m2[:], mx[0:1, 0 : len(chs)], axis=AX.X)
        neg_m2 = pc.tile([1, 1], F32, name="neg_m2")
        nc.vector.reduce_max(neg_m2[:], mx[0:1, 0 : len(chs)], axis=AX.X, negate=True)
        for ci, (c0, cn) in enumerate(chs):
            escr = pc.tile([1, 512], F32, name="escr", tag="escr", bufs=2)
            nc.scalar.activation(
                escr[0:1, 0:cn],
                partial[0:1, c0 : c0 + cn],
                AF.Exp,
                bias=neg_m2[:],
                accum_out=sx[0:1, ci : ci + 1],
            )
        S2 = pc.tile([1, 1], F32, name="S2")
        nc.vector.reduce_sum(S2[:], sx[0:1, 0 : len(chs)], axis=AX.X)
        st2 = pc.tile([1, 8], F32, name="st2")
        nc.vector.memset(st2[:], 0.0)
        nc.vector.tensor_copy(st2[0:1, 0:1], m2[:])
        nc.vector.tensor_copy(st2[0:1, 1:2], S2[:])

        cc2_in = pdram.tile([1, 8], F32, name="cc2_in")
        cc2_out = pdram.tile([8, 8], F32, name="cc2_out", addr_space="Shared")
        nc.scalar.dma_start(cc2_in[:], st2[:])
        nc.gpsimd.collective_compute(
            "AllGather",
            ALU.bypass,
            replica_groups=RG,
            ins=[cc2_in.opt()],
            outs=[cc2_out.opt()],
        )
        ms2 = pc.tile([1, 8], F32, name="ms2")
        nc.scalar.dma_start(ms2[:], cc2_out.rearrange("r c -> c r")[0:1, 0:8])
        ss2 = pc.tile([1, 8], F32, name="ss2")
        nc.scalar.dma_start(ss2[:], cc2_out.rearrange("r c -> c r")[1:2, 0:8])

        negMg2 = pc.tile([1, 1], F32, name="negMg2")
        nc.vector.reduce_max(negMg2[:], ms2[:], axis=AX.X, negate=True)
        e2 = pc.tile([1, 8], F32, name="e2")
        nc.scalar.activation(e2[:], ms2[:], AF.Exp, bias=negMg2[:])
        t2 = pc.tile([1, 8], F32, name="t2")
        nc.vector.tensor_mul(t2[:], e2[:], ss2[:])
        Sg2 = pc.tile([1, 1], F32, name="Sg2")
        nc.vector.reduce_sum(Sg2[:], t2[:], axis=AX.X)
        lnS = pc.tile([1, 1], F32, name="lnS")
        nc.scalar.activation(lnS[:], Sg2[:], AF.Ln)
        nlnS = pc.tile([1, 1], F32, name="nlnS")
        nc.vector.tensor_scalar_mul(nlnS[:], lnS[:], -1.0)
        nshift = pc.tile([1, 1], F32, name="nshift")
        nc.vector.tensor_add(nshift[:], negMg2[:], nlnS[:])

        nc.vector.tensor_scalar_add(
            partial[0:1, 0:vs], partial[0:1, 0:vs], nshift[:]
        )
        nc.scalar.dma_start(o_lsm[:, :], partial[0:1, 0:vs])

    nc.compile()
    return nc


def make_in_maps(inputs, vs=VS, vfull=V):
    """Shard the full inputs into 8 per-core input maps."""
    f32 = lambda x: np.ascontiguousarray(np.asarray(x), dtype=np.float32)

    tok = int(np.asarray(inputs["input_tok"]).reshape(-1)[0])
    W_emb = np.asarray(inputs["W_emb"])
    emb_row = f32(W_emb[tok])                       # [H]
    b_emb = f32(inputs["b_emb"])
    hidden = f32(inputs["hidden"]).reshape(H)
    cell = f32(inputs["cell_state"]).reshape(H)
    enc = f32(np.asarray(inputs["encoder_outputs"])[:, 0, :])   # [L, 2H]
    W_attn = f32(inputs["W_attn"])
    W_attn_out = f32(inputs["W_attn_out"]).reshape(H)
    W_out = np.asarray(inputs["W_out"])
    b_out = f32(inputs["b_out"]).reshape(-1)[:vfull]
    Wg_full = [f32(inputs[n]) for n in ("W_i", "W_o", "W_f", "W_c")]
    b_g = [f32(inputs[n]) for n in ("b_i", "b_o", "b_f", "b_c")]

    vecs = np.zeros((128, 32), np.float32)
    vecs[:, 0:8] = emb_row.reshape(8, 128).T
    vecs[:, 8:16] = b_emb.reshape(8, 128).T
    vecs[:, 16:24] = hidden.reshape(8, 128).T
    vecs[:, 24:32] = W_attn_out.reshape(8, 128).T
    ident = np.eye(128, dtype=np.float32)

    in_maps = []
    for r in range(NCORES):
        c0, c1 = r * vs, (r + 1) * vs
        if c1 <= vfull:
            w_sh = np.ascontiguousarray(W_out[:, c0:c1], dtype=np.float32)
            b_sh = b_out[c0:c1].reshape(1, vs).copy()
        else:
            w_sh = np.zeros((4 * H, vs), np.float32)
            b_sh = np.full((1, vs), NEG_BIG, np.float32)
            if c0 < vfull:
                w_sh[:, : vfull - c0] = W_out[:, c0:vfull]
                b_sh[0, : vfull - c0] = b_out[c0:vfull]
        rows = np.zeros((1, 1664), np.float32)
        rows[0, 0:1024] = f32(inputs["b_attn"])
        for g in range(4):
            rows[0, 1024 + g * 128 : 1024 + (g + 1) * 128] = b_g[g][
                r * 128 : (r + 1) * 128
            ]
        rows[0, 1536:1664] = cell[r * 128 : (r + 1) * 128]
        wg = np.stack(
            [w[:, r * 128 : (r + 1) * 128] for w in Wg_full], axis=0
        ).astype(np.float32)
        in_maps.append(
            {
                "enc": np.ascontiguousarray(enc[r * LS : (r + 1) * LS]),
                "w_attn": W_attn,
                "w_out": w_sh,
                "b_out": b_sh,
                "w_g": np.ascontiguousarray(wg),
                "vecs": vecs,
                "rows": rows,
                "ident": ident,
            }
        )
    return in_maps


def assemble_outputs(results, vs=VS, vfull=V):
    lsm = np.concatenate([results[r]["o_lsm"][0] for r in range(NCORES)])[:vfull]
    h = np.concatenate([results[r]["o_h"][0] for r in range(NCORES)])
    c = np.concatenate([results[r]["o_c"][0] for r in range(NCORES)])
    aw = np.concatenate([results[r]["o_aw"][0] for r in range(NCORES)])
    return (
        lsm.reshape(1, vfull).astype(np.float32),
        h.reshape(1, H).astype(np.float32),
        aw.reshape(L, 1).astype(np.float32),
        c.reshape(1, H).astype(np.float32),
    )


_PROGRAM = None


def _program():
    global _PROGRAM
    if _PROGRAM is None:
        _PROGRAM = build_program()
    return _PROGRAM


def kernel(**inputs):
    nc = _program()
    in_maps = make_in_maps(inputs)
    res = run_bass_kernel_spmd(nc, in_maps, core_ids=list(range(NCORES)))
    return assemble_outputs(res.results)


# revision 24
# speedup vs baseline: 1.1909x; 1.1909x over previous
"""AttnDecoderRNN single-step kernel for 8 Trainium2 NeuronCores.

Full (unsharded) inputs in, full outputs out.  Internally:
  - attention: L=2048 encoder rows sharded 256/core; softmax + context
    combined with a single AllGather of (local max, local sumexp, partial
    context) per core.
  - LSTM: gate outputs column-sharded 128/core.
  - output layer: V=50257 column-sharded 6283/core (V padded to 50264);
    the K=4096 contraction is split into [emb|hidden] rows (ready at t=0)
    and [context] rows (ready after the AllGather) so the big W_out DMA
    stream never stalls.
  - log-softmax: local max/sumexp per V-shard + a second tiny AllGather.
All math is fp32 on device; the host only routes/reshapes data.
"""

import os
import sys

import numpy as np

if "/opt/trn_rl_repo" not in sys.path:
    sys.path.insert(0, "/opt/trn_rl_repo")

import concourse.bacc as bacc
import concourse.bass as bass
import concourse.mybir as mybir
import concourse.tile as tile
from concourse.bass_utils import run_bass_kernel_spmd

NCORES = 8
H = 1024
V = 50257
L = 2048
LS = L // NCORES            # 256 encoder rows per core
VS = 6284                   # padded vocab shard (8*6284 = 50272 >= V; even
                            # chunk widths — fp32r needs even moving free dims)
NEG_BIG = -1.0e30

F32 = mybir.dt.float32
F32R = mybir.dt.float32r
AF = mybir.ActivationFunctionType
ALU = mybir.AluOpType
AX = mybir.AxisListType
RG = [list(range(NCORES))]


def _chunks(n, step=512):
    out = []
    c0 = 0
    while c0 < n:
        out.append((c0, min(step, n - c0)))
        c0 += step
    return out


def build_program(vs=VS, no_cc=False, no_out=False, no_attn=False):
    """Build + compile the SPMD Bass program (identical on all 8 cores)."""
    from contextlib import ExitStack

    nc = bacc.Bacc(
        "TRN2",
        target_bir_lowering=False,
        debug=False,
        num_devices=NCORES,
    )

    # ---- I/O ----
    enc_d = nc.dram_tensor("enc", [LS, 2 * H], F32, kind="ExternalInput")
    wattn_d = nc.dram_tensor("w_attn", [3 * H, H], F32R, kind="ExternalInput")
    wout_d = nc.dram_tensor("w_out", [4 * H, vs], F32R, kind="ExternalInput")
    bout_d = nc.dram_tensor("b_out", [1, vs], F32, kind="ExternalInput")
    wg_d = nc.dram_tensor("w_g", [4, 4 * H, 128], F32, kind="ExternalInput")
    vecs_d = nc.dram_tensor("vecs", [128, 32], F32R, kind="ExternalInput")
    rows_d = nc.dram_tensor("rows", [1, 1664], F32, kind="ExternalInput")
    ident_d = nc.dram_tensor("ident", [128, 128], F32, kind="ExternalInput")

    o_lsm = nc.dram_tensor("o_lsm", [1, vs], F32, kind="ExternalOutput")
    o_h = nc.dram_tensor("o_h", [1, 128], F32, kind="ExternalOutput")
    o_c = nc.dram_tensor("o_c", [1, 128], F32, kind="ExternalOutput")
    o_aw = nc.dram_tensor("o_aw", [1, LS], F32, kind="ExternalOutput")

    chs = _chunks(vs)
    npart = 512 * len(chs)  # partial row width (512-aligned slots per chunk)

    with tile.TileContext(nc) as tc, ExitStack() as ctx:
        pc = ctx.enter_context(tc.tile_pool(name="const", bufs=1))
        pw_out = ctx.enter_context(tc.tile_pool(name="pwout", bufs=2))
        pw_attn = ctx.enter_context(tc.tile_pool(name="pwattn", bufs=2))
        pw_misc = ctx.enter_context(tc.tile_pool(name="pwmisc", bufs=2))
        ps_a = ctx.enter_context(tc.tile_pool(name="psA", bufs=2, space="PSUM"))
        ps_t = ctx.enter_context(tc.tile_pool(name="psT", bufs=2, space="PSUM"))
        ps_r = ctx.enter_context(tc.tile_pool(name="psR", bufs=3, space="PSUM"))
        pdram = ctx.enter_context(tc.tile_pool(name="dram", bufs=1, space="DRAM"))

        # ---- constants / small vectors ----
        vecs_sb = pc.tile([128, 32], F32R, name="vecs_sb")
        nc.scalar.dma_start(vecs_sb[:], vecs_d[:, :])
        ident_sb = pc.tile([128, 128], F32, name="ident_sb")
        nc.scalar.dma_start(ident_sb[:], ident_d[:, :])
        rows_sb = pc.tile([1, 1664], F32, name="rows_sb")
        nc.scalar.dma_start(rows_sb[:], rows_d[:, :])
        ones_sb = pc.tile([1, 1], F32, name="ones_sb")
        nc.vector.memset(ones_sb[:], 1.0)

        # emb (+ b_emb) in transposed column layout [128, 8]
        emb_f = pc.tile([128, 8], F32R, name="emb_f")
        nc.vector.tensor_add(emb_f[:], vecs_sb[:, 0:8], vecs_sb[:, 8:16])
        hidT = vecs_sb[:, 16:24]     # hidden.T columns
        waoT = vecs_sb[:, 24:32]     # W_attn_out as 8 lhsT columns [128,1]

        # ---- encoder rows: natural + transposed ----
        # enc_sb[:, t*2H + c] = enc[t*128 + p, c]   (t = 0,1)
        enc_sb = pc.tile([128, 2 * 2 * H], F32, name="enc_sb")
        nc.scalar.dma_start(
            enc_sb[:], enc_d.ap().rearrange("(t p) c -> p t c", p=128)
        )
        # encT[:, k*LS + t*128 + l] = enc[t*128 + l, k*128 + p]
        encT = pc.tile([128, 16 * LS], F32R, name="encT")
        if no_attn:
            nc.vector.memset(encT[:], 0.01)
        for t in range(2 if not no_attn else 0):
            for k in range(16):
                ps = ps_t.tile([128, 128], F32, name="tp_ps", tag="psT")
                nc.tensor.transpose(
                    ps[:],
                    enc_sb[:, t * 2 * H + k * 128 : t * 2 * H + (k + 1) * 128],
                    ident_sb[:],
                )
                nc.vector.tensor_copy(
                    encT[:, k * LS + t * 128 : k * LS + (t + 1) * 128], ps[:]
                )

        # ---- h_row = hidden @ W_attn[2H:3H, :] + b_attn  -> [1, 1024] ----
        hrow_sb = pc.tile([1, H], F32, name="hrow_sb")
        for n in range(4):
            wh = pw_misc.tile([128, 8 * 256], F32R, name="wh", tag="wmisc")
            nc.scalar.dma_start(
                wh[:],
                wattn_d[2 * H : 3 * H, n * 256 : (n + 1) * 256]
                .rearrange("(k p) c -> p k c", p=128),
            )
            psh = ps_r.tile([1, 512], F32, name="psh", tag="psR")
            for k in range(8):
                nc.tensor.matmul(
                    psh[0:1, 0:256],
                    hidT[:, k : k + 1],
                    wh[:, k * 256 : (k + 1) * 256],
                    start=(k == 0),
                    stop=(k == 7),
                )
            # += b_attn chunk (on DVE, keeping the matmul group pure fp32r)
            nc.vector.tensor_add(
                hrow_sb[0:1, n * 256 : (n + 1) * 256],
                psh[0:1, 0:256],
                rows_sb[0:1, n * 256 : (n + 1) * 256],
            )

        # transpose h_row -> per-feature bias columns biasT [128, 8]
        biasT = pc.tile([128, 8], F32, name="biasT")
        for m in range(8):
            ps = ps_t.tile([128, 1], F32, name="r2c_ps", tag="psT")
            nc.tensor.matmul(
                ps[:],
                hrow_sb[0:1, m * 128 : (m + 1) * 128],
                ones_sb[:],
                start=True,
                stop=True,
            )
            nc.vector.tensor_copy(biasT[:, m : m + 1], ps[:])

        # ---- scores: tanh(enc @ W1 + h_row + b_attn) @ W_attn_out ----
        score_ps = ps_r.tile([1, LS], F32, name="score_ps", tag="psR")
        if no_attn:
            nc.tensor.matmul(score_ps[:], waoT[:, 0:1], encT[:, 0:LS],
                             start=True, stop=True)
        for m in range(8 if not no_attn else 0):
            wa = pw_attn.tile([128, 16 * 128], F32R, name="wa", tag="wattn")
            nc.scalar.dma_start(
                wa[:],
                wattn_d[0 : 2 * H, m * 128 : (m + 1) * 128]
                .rearrange("(k p) c -> p k c", p=128),
            )
            pe = ps_a.tile([128, LS], F32, name="pe", tag="psA")
            for k in range(16):
                nc.tensor.matmul(
                    pe[:],
                    wa[:, k * 128 : (k + 1) * 128],
                    encT[:, k * LS : (k + 1) * LS],
                    start=(k == 0),
                    stop=(k == 15),
                )
            th = pc.tile([128, LS], F32R, name="th", tag="tanhT", bufs=2)
            nc.scalar.activation(th[:], pe[:], AF.Tanh, bias=biasT[:, m : m + 1])
            nc.tensor.matmul(
                score_ps[:],
                waoT[:, m : m + 1],
                th[:],
                start=(m == 0),
                stop=(m == 7),
            )

        # ---- local softmax stats + partial context ----
        m_l = pc.tile([1, 1], F32, name="m_l")
        nc.vector.reduce_max(m_l[:], score_ps[:], axis=AX.X)
        neg_ml = pc.tile([1, 1], F32, name="neg_ml")
        nc.vector.reduce_max(neg_ml[:], score_ps[:], axis=AX.X, negate=True)
        e_row = pc.tile([1, LS], F32, name="e_row")
        S_l = pc.tile([1, 1], F32, name="S_l")
        nc.scalar.activation(
            e_row[:], score_ps[:], AF.Exp, bias=neg_ml[:], accum_out=S_l[:]
        )
        # e as columns for the context matmul
        ecol = pc.tile([128, 2], F32, name="ecol")
        for j in range(2):
            ps = ps_t.tile([128, 1], F32, name="ec_ps", tag="psT")
            nc.tensor.matmul(
                ps[:],
                e_row[0:1, j * 128 : (j + 1) * 128],
                ones_sb[:],
                start=True,
                stop=True,
            )
            nc.vector.tensor_copy(ecol[:, j : j + 1], ps[:])

        stats = pc.tile([1, 2056], F32, name="stats")
        nc.vector.tensor_copy(stats[0:1, 0:1], m_l[:])
        nc.vector.tensor_copy(stats[0:1, 1:2], S_l[:])
        nc.vector.memset(stats[0:1, 2:8], 0.0)
        for n in range(4):
            ps = ps_r.tile([1, 512], F32, name="ctxp_ps", tag="psR")
            for j in range(2):
                nc.tensor.matmul(
                    ps[:],
                    ecol[:, j : j + 1],
                    enc_sb[:, j * 2 * H + n * 512 : j * 2 * H + (n + 1) * 512],
                    start=(j == 0),
                    stop=(j == 1),
                )
            nc.vector.tensor_copy(stats[0:1, 8 + n * 512 : 8 + (n + 1) * 512], ps[:])

        # ---- AllGather #1: (max, sumexp, partial context) ----
        cc1_in = pdram.tile([1, 2056], F32, name="cc1_in")
        cc1_out = pdram.tile([8, 2056], F32, name="cc1_out", addr_space="Local" if no_cc else "Shared")
        nc.scalar.dma_start(cc1_in[:], stats[:])
        if no_cc:
            for _r in range(NCORES):
                nc.scalar.dma_start(cc1_out[_r : _r + 1, :], cc1_in[:])
        else:
            nc.gpsimd.collective_compute(
                "AllGather",
                ALU.bypass,
                replica_groups=RG,
                ins=[cc1_in.opt()],
                outs=[cc1_out.opt()],
            )

        # ---- output layer phase 1 (emb|hidden rows) — runs while AG1 is in
        # flight; emitted here so PE's in-order stream isn't blocked behind
        # anything that depends on the collective.
        # partial holds phase-1 sums per 512-slot, then (slots being
        # contiguous for every chunk but the ragged last) the final out row.
        partial = pc.tile([1, npart], F32, name="partial")
        out_chs = [] if no_out else chs
        if no_out:
            nc.vector.memset(partial[:], 0.0)

        # phase 1: K rows 0:2048 ([emb | hidden]) — no ctx dependency
        for ci, (c0, cn) in enumerate(out_chs):
            w1 = pw_out.tile([128, 16 * 512], F32R, name="w1", tag="wout")
            nc.sync.dma_start(
                w1[:, 0 : 16 * cn],
                wout_d[0 : 2 * H, c0 : c0 + cn].rearrange(
                    "(k p) c -> p k c", p=128
                ),
            )
            ps = ps_r.tile([1, 512], F32, name="po1", tag="psR")
            for k in range(16):
                lhs = emb_f[:, k : k + 1] if k < 8 else hidT[:, k - 8 : k - 7]
                nc.tensor.matmul(
                    ps[0:1, 0:cn],
                    lhs,
                    w1[:, k * cn : (k + 1) * cn],
                    start=(k == 0),
                    stop=(k == 15),
                )
            nc.vector.tensor_copy(
                partial[0:1, ci * 512 : ci * 512 + cn], ps[0:1, 0:cn]
            )

        crows = pc.tile([8, 2 * H], F32, name="crows")
        nc.scalar.dma_start(crows[:], cc1_out[0:8, 8:2056])
        ms1 = pc.tile([1, 8], F32, name="ms1")
        nc.scalar.dma_start(ms1[:], cc1_out.rearrange("r c -> c r")[0:1, 0:8])
        ss1 = pc.tile([1, 8], F32, name="ss1")
        nc.scalar.dma_start(ss1[:], cc1_out.rearrange("r c -> c r")[1:2, 0:8])

        # global combine (all on partition 0)
        negMg = pc.tile([1, 1], F32, name="negMg")
        nc.vector.reduce_max(negMg[:], ms1[:], axis=AX.X, negate=True)
        sc1 = pc.tile([1, 8], F32, name="sc1")
        nc.scalar.activation(sc1[:], ms1[:], AF.Exp, bias=negMg[:])
        t1 = pc.tile([1, 8], F32, name="t1")
        nc.vector.tensor_mul(t1[:], sc1[:], ss1[:])
        S_tot = pc.tile([1, 1], F32, name="S_tot")
        nc.vector.reduce_sum(S_tot[:], t1[:], axis=AX.X)
        rinv = pc.tile([1, 1], F32, name="rinv")
        nc.vector.reciprocal(rinv[:], S_tot[:])
        sc2 = pc.tile([1, 8], F32, name="sc2")
        nc.vector.tensor_scalar_mul(sc2[:], sc1[:], rinv[:])

        # attn_weights output for this core's L-shard
        aw_s = pc.tile([1, 1], F32, name="aw_s")
        nc.scalar.activation(aw_s[:], m_l[:], AF.Exp, bias=negMg[:])
        aw_s2 = pc.tile([1, 1], F32, name="aw_s2")
        nc.vector.tensor_mul(aw_s2[:], aw_s[:], rinv[:])
        aw_row = pc.tile([1, LS], F32, name="aw_row")
        nc.vector.tensor_scalar_mul(aw_row[:], e_row[:], aw_s2[:])
        nc.scalar.dma_start(o_aw[:, :], aw_row[:])

        # context = sc2 @ crows  -> [1, 2048]
        sc2c_ps = ps_t.tile([8, 1], F32, name="sc2c_ps", tag="psT")
        nc.tensor.matmul(sc2c_ps[:], sc2[:], ones_sb[:], start=True, stop=True)
        sc2col = pc.tile([8, 1], F32, name="sc2col")
        nc.vector.tensor_copy(sc2col[:], sc2c_ps[:])
        ctx_row = pc.tile([1, 2 * H], F32, name="ctx_row")
        for n in range(4):
            ps = ps_r.tile([1, 512], F32, name="ctx_ps", tag="psR")
            nc.tensor.matmul(
                ps[:],
                sc2col[:],
                crows[:, n * 512 : (n + 1) * 512],
                start=True,
                stop=True,
            )
            nc.vector.tensor_copy(ctx_row[0:1, n * 512 : (n + 1) * 512], ps[:])

        # unrounded fp32 copies of [emb | ctx | hidden] columns for the LSTM
        # (keeps the gates at full fp32 precision; fp32r wouldn't speed up
        # N=128 matmuls anyway)
        xT32 = pc.tile([128, 32], F32, name="xT32")
        nc.vector.tensor_add(xT32[:, 0:8], vecs_sb[:, 0:8], vecs_sb[:, 8:16])
        nc.vector.tensor_copy(xT32[:, 24:32], vecs_sb[:, 16:24])

        # context transposed columns ctxT [128, 16]
        ctxT = pc.tile([128, 16], F32R, name="ctxT")
        for t in range(16):
            ps = ps_t.tile([128, 1], F32, name="ctxT_ps", tag="psT")
            nc.tensor.matmul(
                ps[:],
                ctx_row[0:1, t * 128 : (t + 1) * 128],
                ones_sb[:],
                start=True,
                stop=True,
            )
            nc.vector.tensor_copy(ctxT[:, t : t + 1], ps[:])
            nc.vector.tensor_copy(xT32[:, 8 + t : 9 + t], ps[:])

        def xT_lstm(k):
            # LSTM input order: [emb, context, hidden]
            return xT32[:, k : k + 1]

        # ---- LSTM gates (this core's 128 output columns) ----
        gates = []
        for g in range(4):
            psg = ps_r.tile([1, 128], F32, name="psg", tag="psR")
            for half in range(2):
                wg = pw_misc.tile([128, 16 * 128], F32, name="wg", tag="wmisc")
                nc.scalar.dma_start(
                    wg[:],
                    wg_d[g][half * 2 * H : (half + 1) * 2 * H, :]
                    .rearrange("(k p) c -> p k c", p=128),
                )
                for kk in range(16):
                    k = half * 16 + kk
                    nc.tensor.matmul(
                        psg[:],
                        xT_lstm(k),
                        wg[:, kk * 128 : (kk + 1) * 128],
                        start=(k == 0),
                        stop=(k == 31),
                    )
            gsum = pc.tile([1, 128], F32, name=f"gsum{g}", tag=f"gsum{g}")
            nc.vector.tensor_add(
                gsum[:], psg[:], rows_sb[0:1, 1024 + g * 128 : 1024 + (g + 1) * 128]
            )
            gact = pc.tile([1, 128], F32, name=f"gact{g}", tag=f"gact{g}")
            nc.scalar.activation(
                gact[:], gsum[:], AF.Tanh if g == 3 else AF.Sigmoid
            )
            gates.append(gact)
        i_g, o_g, f_g, c_g = gates

        fc = pc.tile([1, 128], F32, name="fc")
        nc.vector.tensor_mul(fc[:], f_g[:], rows_sb[0:1, 1536:1664])
        ic = pc.tile([1, 128], F32, name="ic")
        nc.vector.tensor_mul(ic[:], i_g[:], c_g[:])
        cnew = pc.tile([1, 128], F32, name="cnew")
        nc.vector.tensor_add(cnew[:], fc[:], ic[:])
        nc.scalar.dma_start(o_c[:, :], cnew[:])
        ctanh = pc.tile([1, 128], F32, name="ctanh")
        nc.scalar.activation(ctanh[:], cnew[:], AF.Tanh)
        hnew = pc.tile([1, 128], F32, name="hnew")
        nc.vector.tensor_mul(hnew[:], o_g[:], ctanh[:])
        nc.scalar.dma_start(o_h[:, :], hnew[:])

        # ---- output layer: out = [emb|hidden|ctx] @ W_out + b_out ----

        # phase 2: K rows 2048:4096 (context) + finalize
        for ci, (c0, cn) in enumerate(out_chs):
            w2 = pw_out.tile([128, 16 * 512], F32R, name="w2", tag="wout")
            nc.sync.dma_start(
                w2[:, 0 : 16 * cn],
                wout_d[2 * H : 4 * H, c0 : c0 + cn].rearrange(
                    "(k p) c -> p k c", p=128
                ),
            )
            ps = ps_r.tile([1, 512], F32, name="po2", tag="psR")
            for k in range(16):
                nc.tensor.matmul(
                    ps[0:1, 0:cn],
                    ctxT[:, k : k + 1],
                    w2[:, k * cn : (k + 1) * cn],
                    start=(k == 0),
                    stop=(k == 15),
                )
            bout_c = pc.tile([1, 512], F32, name="bout_c", tag="boutc", bufs=2)
            nc.scalar.dma_start(bout_c[0:1, 0:cn], bout_d[0:1, c0 : c0 + cn])
            nc.vector.tensor_add(
                ps[0:1, 0:cn], ps[0:1, 0:cn], partial[0:1, ci * 512 : ci * 512 + cn]
            )
            nc.vector.tensor_add(
                partial[0:1, c0 : c0 + cn], ps[0:1, 0:cn], bout_c[0:1, 0:cn]
            )

        # ---- log-softmax (chunked local max / sum-exp over the out row) ----
        mx = pc.tile([1, 16], F32, name="mx")
        sx = pc.tile([1, 16], F32, name="sx")
        for ci, (c0, cn) in enumerate(chs):
            nc.vector.reduce_max(
                mx[0:1, ci : ci + 1], partial[0:1, c0 : c0 + cn], axis=AX.X
            )
        m2 = pc.tile([1, 1], F32, name="m2")
        nc.vector.reduce_max(m2[:], mx[0:1, 0 : len(chs)], axis=AX.X)
        neg_m2 = pc.tile([1, 1], F32, name="neg_m2")
        nc.vector.reduce_max(neg_m2[:], mx[0:1, 0 : len(chs)], axis=AX.X, negate=True)
        for ci, (c0, cn) in enumerate(chs):
            escr = pc.tile([1, 512], F32, name="escr", tag="escr", bufs=1)
            nc.scalar.activation(
                escr[0:1, 0:cn],
                partial[0:1, c0 : c0 + cn],
                AF.Exp,
                bias=neg_m2[:],
                accum_out=sx[0:1, ci : ci + 1],
            )
        S2 = pc.tile([1, 1], F32, name="S2")
        nc.vector.reduce_sum(S2[:], sx[0:1, 0 : len(chs)], axis=AX.X)
        st2 = pc.tile([1, 8], F32, name="st2")
        nc.vector.memset(st2[:], 0.0)
        nc.vector.tensor_copy(st2[0:1, 0:1], m2[:])
        nc.vector.tensor_copy(st2[0:1, 1:2], S2[:])

        cc2_in = pdram.tile([1, 8], F32, name="cc2_in")
        cc2_out = pdram.tile([8, 8], F32, name="cc2_out", addr_space="Local" if no_cc else "Shared")
        nc.scalar.dma_start(cc2_in[:], st2[:])
        if no_cc:
            for _r in range(NCORES):
                nc.scalar.dma_start(cc2_out[_r : _r + 1, :], cc2_in[:])
        else:
            nc.gpsimd.collective_compute(
                "AllGather",
                ALU.bypass,
                replica_groups=RG,
                ins=[cc2_in.opt()],
                outs=[cc2_out.opt()],
            )
        ms2 = pc.tile([1, 8], F32, name="ms2")
        nc.scalar.dma_start(ms2[:], cc2_out.rearrange("r c -> c r")[0:1, 0:8])
        ss2 = pc.tile([1, 8], F32, name="ss2")
        nc.scalar.dma_start(ss2[:], cc2_out.rearrange("r c -> c r")[1:2, 0:8])

        negMg2 = pc.tile([1, 1], F32, name="negMg2")
        nc.vector.reduce_max(negMg2[:], ms2[:], axis=AX.X, negate=True)
        e2 = pc.tile([1, 8], F32, name="e2")
        nc.scalar.activation(e2[:], ms2[:], AF.Exp, bias=negMg2[:])
        t2 = pc.tile([1, 8], F32, name="t2")
        nc.vector.tensor_mul(t2[:], e2[:], ss2[:])
        Sg2 = pc.tile([1, 1], F32, name="Sg2")
        nc.vector.reduce_sum(Sg2[:], t2[:], axis=AX.X)
        lnS = pc.tile([1, 1], F32, name="lnS")
        nc.scalar.activation(lnS[:], Sg2[:], AF.Ln)
        nlnS = pc.tile([1, 1], F32, name="nlnS")
        nc.vector.tensor_scalar_mul(nlnS[:], lnS[:], -1.0)
        nshift = pc.tile([1, 1], F32, name="nshift")
        nc.vector.tensor_add(nshift[:], negMg2[:], nlnS[:])

        nc.vector.tensor_scalar_add(
            partial[0:1, 0:vs], partial[0:1, 0:vs], nshift[:]
        )
        nc.scalar.dma_start(o_lsm[:, :], partial[0:1, 0:vs])

    nc.compile()
    return nc


def make_in_maps(inputs, vs=VS, vfull=V):
    """Shard the full inputs into 8 per-core input maps."""
    f32 = lambda x: np.ascontiguousarray(np.asarray(x), dtype=np.float32)

    tok = int(np.asarray(inputs["input_tok"]).reshape(-1)[0])
    W_emb = np.asarray(inputs["W_emb"])
    emb_row = f32(W_emb[tok])                       # [H]
    b_emb = f32(inputs["b_emb"])
    hidden = f32(inputs["hidden"]).reshape(H)
    cell = f32(inputs["cell_state"]).reshape(H)
    enc = f32(np.asarray(inputs["encoder_outputs"])[:, 0, :])   # [L, 2H]
    W_attn = f32(inputs["W_attn"])
    W_attn_out = f32(inputs["W_attn_out"]).reshape(H)
    W_out = np.asarray(inputs["W_out"])
    b_out = f32(inputs["b_out"]).reshape(-1)[:vfull]
    Wg_full = [f32(inputs[n]) for n in ("W_i", "W_o", "W_f", "W_c")]
    b_g = [f32(inputs[n]) for n in ("b_i", "b_o", "b_f", "b_c")]

    vecs = np.zeros((128, 32), np.float32)
    vecs[:, 0:8] = emb_row.reshape(8, 128).T
    vecs[:, 8:16] = b_emb.reshape(8, 128).T
    vecs[:, 16:24] = hidden.reshape(8, 128).T
    vecs[:, 24:32] = W_attn_out.reshape(8, 128).T
    ident = np.eye(128, dtype=np.float32)

    in_maps = []
    for r in range(NCORES):
        c0, c1 = r * vs, (r + 1) * vs
        if c1 <= vfull:
            w_sh = np.ascontiguousarray(W_out[:, c0:c1], dtype=np.float32)
            b_sh = b_out[c0:c1].reshape(1, vs).copy()
        else:
            w_sh = np.zeros((4 * H, vs), np.float32)
            b_sh = np.full((1, vs), NEG_BIG, np.float32)
            if c0 < vfull:
                w_sh[:, : vfull - c0] = W_out[:, c0:vfull]
                b_sh[0, : vfull - c0] = b_out[c0:vfull]
        rows = np.zeros((1, 1664), np.float32)
        rows[0, 0:1024] = f32(inputs["b_attn"])
        for g in range(4):
            rows[0, 1024 + g * 128 : 1024 + (g + 1) * 128] = b_g[g][
                r * 128 : (r + 1) * 128
            ]
        rows[0, 1536:1664] = cell[r * 128 : (r + 1) * 128]
        wg = np.stack(
            [w[:, r * 128 : (r + 1) * 128] for w in Wg_full], axis=0
        ).astype(np.float32)
        in_maps.append(
            {
                "enc": np.ascontiguousarray(enc[r * LS : (r + 1) * LS]),
                "w_attn": W_attn,
                "w_out": w_sh,
                "b_out": b_sh,
                "w_g": np.ascontiguousarray(wg),
                "vecs": vecs,
                "rows": rows,
                "ident": ident,
            }
        )
    return in_maps


def assemble_outputs(results, vs=VS, vfull=V):
    lsm = np.concatenate([results[r]["o_lsm"][0] for r in range(NCORES)])[:vfull]
    h = np.concatenate([results[r]["o_h"][0] for r in range(NCORES)])
    c = np.concatenate([results[r]["o_c"][0] for r in range(NCORES)])
    aw = np.concatenate([results[r]["o_aw"][0] for r in range(NCORES)])
    return (
        lsm.reshape(1, vfull).astype(np.float32),
        h.reshape(1, H).astype(np.float32),
        aw.reshape(L, 1).astype(np.float32),
        c.reshape(1, H).astype(np.float32),
    )


_PROGRAM = None


def _program():
    global _PROGRAM
    if _PROGRAM is None:
        _PROGRAM = build_program()
    return _PROGRAM


def kernel(**inputs):
    nc = _program()
    in_maps = make_in_maps(inputs)
    res = run_bass_kernel_spmd(nc, in_maps, core_ids=list(range(NCORES)))
    return assemble_outputs(res.results)


# revision 25
# speedup vs baseline: 1.1927x; 1.0015x over previous
"""AttnDecoderRNN single-step kernel for 8 Trainium2 NeuronCores.

Full (unsharded) inputs in, full outputs out.  Internally:
  - attention: L=2048 encoder rows sharded 256/core; softmax + context
    combined with a single AllGather of (local max, local sumexp, partial
    context) per core.
  - LSTM: gate outputs column-sharded 128/core.
  - output layer: V=50257 column-sharded 6283/core (V padded to 50264);
    the K=4096 contraction is split into [emb|hidden] rows (ready at t=0)
    and [context] rows (ready after the AllGather) so the big W_out DMA
    stream never stalls.
  - log-softmax: local max/sumexp per V-shard + a second tiny AllGather.
All math is fp32 on device; the host only routes/reshapes data.
"""

import os
import sys

import numpy as np

if "/opt/trn_rl_repo" not in sys.path:
    sys.path.insert(0, "/opt/trn_rl_repo")

import concourse.bacc as bacc
import concourse.bass as bass
import concourse.mybir as mybir
import concourse.tile as tile
from concourse.bass_utils import run_bass_kernel_spmd

NCORES = 8
H = 1024
V = 50257
L = 2048
LS = L // NCORES            # 256 encoder rows per core
VS = 6284                   # padded vocab shard (8*6284 = 50272 >= V; even
                            # chunk widths — fp32r needs even moving free dims)
NEG_BIG = -1.0e30

F32 = mybir.dt.float32
F32R = mybir.dt.float32r
AF = mybir.ActivationFunctionType
ALU = mybir.AluOpType
AX = mybir.AxisListType
RG = [list(range(NCORES))]


def _chunks(n, step=512):
    out = []
    c0 = 0
    while c0 < n:
        out.append((c0, min(step, n - c0)))
        c0 += step
    return out


def build_program(vs=VS, no_cc=False, no_out=False, no_attn=False):
    """Build + compile the SPMD Bass program (identical on all 8 cores)."""
    from contextlib import ExitStack

    nc = bacc.Bacc(
        "TRN2",
        target_bir_lowering=False,
        debug=False,
        num_devices=NCORES,
    )

    # ---- I/O ----
    enc_d = nc.dram_tensor("enc", [LS, 2 * H], F32, kind="ExternalInput")
    wattn_d = nc.dram_tensor("w_attn", [3 * H, H], F32, kind="ExternalInput")
    wout_d = nc.dram_tensor("w_out", [4 * H, vs], F32R, kind="ExternalInput")
    bout_d = nc.dram_tensor("b_out", [1, vs], F32, kind="ExternalInput")
    wg_d = nc.dram_tensor("w_g", [4, 4 * H, 128], F32, kind="ExternalInput")
    vecs_d = nc.dram_tensor("vecs", [128, 32], F32R, kind="ExternalInput")
    rows_d = nc.dram_tensor("rows", [1, 1664], F32, kind="ExternalInput")
    ident_d = nc.dram_tensor("ident", [128, 128], F32, kind="ExternalInput")

    o_lsm = nc.dram_tensor("o_lsm", [1, vs], F32, kind="ExternalOutput")
    o_h = nc.dram_tensor("o_h", [1, 128], F32, kind="ExternalOutput")
    o_c = nc.dram_tensor("o_c", [1, 128], F32, kind="ExternalOutput")
    o_aw = nc.dram_tensor("o_aw", [1, LS], F32, kind="ExternalOutput")

    chs = _chunks(vs)
    npart = 512 * len(chs)  # partial row width (512-aligned slots per chunk)

    with tile.TileContext(nc) as tc, ExitStack() as ctx:
        pc = ctx.enter_context(tc.tile_pool(name="const", bufs=1))
        pw_out = ctx.enter_context(tc.tile_pool(name="pwout", bufs=2))
        pw_attn = ctx.enter_context(tc.tile_pool(name="pwattn", bufs=2))
        pw_misc = ctx.enter_context(tc.tile_pool(name="pwmisc", bufs=2))
        ps_a = ctx.enter_context(tc.tile_pool(name="psA", bufs=2, space="PSUM"))
        ps_t = ctx.enter_context(tc.tile_pool(name="psT", bufs=2, space="PSUM"))
        ps_r = ctx.enter_context(tc.tile_pool(name="psR", bufs=3, space="PSUM"))
        pdram = ctx.enter_context(tc.tile_pool(name="dram", bufs=1, space="DRAM"))

        # ---- constants / small vectors ----
        vecs_sb = pc.tile([128, 32], F32R, name="vecs_sb")
        nc.scalar.dma_start(vecs_sb[:], vecs_d[:, :])
        ident_sb = pc.tile([128, 128], F32, name="ident_sb")
        nc.scalar.dma_start(ident_sb[:], ident_d[:, :])
        rows_sb = pc.tile([1, 1664], F32, name="rows_sb")
        nc.scalar.dma_start(rows_sb[:], rows_d[:, :])
        ones_sb = pc.tile([1, 1], F32, name="ones_sb")
        nc.vector.memset(ones_sb[:], 1.0)

        # emb (+ b_emb) in transposed column layout [128, 8] (fp32r, for the
        # output-layer lhsT); fp32 copies of hidden.T / W_attn_out.T for the
        # full-precision attention matmuls
        emb_f = pc.tile([128, 8], F32R, name="emb_f")
        nc.vector.tensor_add(emb_f[:], vecs_sb[:, 0:8], vecs_sb[:, 8:16])
        hidT = vecs_sb[:, 16:24]     # hidden.T columns (fp32r)
        hid32 = pc.tile([128, 8], F32, name="hid32")
        nc.vector.tensor_copy(hid32[:], vecs_sb[:, 16:24])
        wao32 = pc.tile([128, 8], F32, name="wao32")
        nc.vector.tensor_copy(wao32[:], vecs_sb[:, 24:32])

        # ---- encoder rows: natural + transposed ----
        # enc_sb[:, t*2H + c] = enc[t*128 + p, c]   (t = 0,1)
        enc_sb = pc.tile([128, 2 * 2 * H], F32, name="enc_sb")
        nc.scalar.dma_start(
            enc_sb[:], enc_d.ap().rearrange("(t p) c -> p t c", p=128)
        )
        # encT[:, k*LS + t*128 + l] = enc[t*128 + l, k*128 + p]
        encT = pc.tile([128, 16 * LS], F32, name="encT")
        if no_attn:
            nc.vector.memset(encT[:], 0.01)
        for t in range(2 if not no_attn else 0):
            for k in range(16):
                ps = ps_t.tile([128, 128], F32, name="tp_ps", tag="psT")
                nc.tensor.transpose(
                    ps[:],
                    enc_sb[:, t * 2 * H + k * 128 : t * 2 * H + (k + 1) * 128],
                    ident_sb[:],
                )
                nc.vector.tensor_copy(
                    encT[:, k * LS + t * 128 : k * LS + (t + 1) * 128], ps[:]
                )

        # ---- h_row = hidden @ W_attn[2H:3H, :] + b_attn  -> [1, 1024] ----
        hrow_sb = pc.tile([1, H], F32, name="hrow_sb")
        for n in range(4):
            wh = pw_misc.tile([128, 8 * 256], F32, name="wh", tag="wmisc")
            nc.scalar.dma_start(
                wh[:],
                wattn_d[2 * H : 3 * H, n * 256 : (n + 1) * 256]
                .rearrange("(k p) c -> p k c", p=128),
            )
            psh = ps_r.tile([1, 512], F32, name="psh", tag="psR")
            for k in range(8):
                nc.tensor.matmul(
                    psh[0:1, 0:256],
                    hid32[:, k : k + 1],
                    wh[:, k * 256 : (k + 1) * 256],
                    start=(k == 0),
                    stop=(k == 7),
                )
            # += b_attn chunk (on DVE, keeping the matmul group pure fp32r)
            nc.vector.tensor_add(
                hrow_sb[0:1, n * 256 : (n + 1) * 256],
                psh[0:1, 0:256],
                rows_sb[0:1, n * 256 : (n + 1) * 256],
            )

        # transpose h_row -> per-feature bias columns biasT [128, 8]
        biasT = pc.tile([128, 8], F32, name="biasT")
        for m in range(8):
            ps = ps_t.tile([128, 1], F32, name="r2c_ps", tag="psT")
            nc.tensor.matmul(
                ps[:],
                hrow_sb[0:1, m * 128 : (m + 1) * 128],
                ones_sb[:],
                start=True,
                stop=True,
            )
            nc.vector.tensor_copy(biasT[:, m : m + 1], ps[:])

        # ---- scores: tanh(enc @ W1 + h_row + b_attn) @ W_attn_out ----
        score_ps = ps_r.tile([1, LS], F32, name="score_ps", tag="psR")
        if no_attn:
            nc.tensor.matmul(score_ps[:], wao32[:, 0:1], encT[:, 0:LS],
                             start=True, stop=True)
        for m in range(8 if not no_attn else 0):
            wa = pw_attn.tile([128, 16 * 128], F32, name="wa", tag="wattn")
            nc.scalar.dma_start(
                wa[:],
                wattn_d[0 : 2 * H, m * 128 : (m + 1) * 128]
                .rearrange("(k p) c -> p k c", p=128),
            )
            pe = ps_a.tile([128, LS], F32, name="pe", tag="psA")
            for k in range(16):
                nc.tensor.matmul(
                    pe[:],
                    wa[:, k * 128 : (k + 1) * 128],
                    encT[:, k * LS : (k + 1) * LS],
                    start=(k == 0),
                    stop=(k == 15),
                )
            th = pc.tile([128, LS], F32, name="th", tag="tanhT", bufs=2)
            nc.scalar.activation(th[:], pe[:], AF.Tanh, bias=biasT[:, m : m + 1])
            nc.tensor.matmul(
                score_ps[:],
                wao32[:, m : m + 1],
                th[:],
                start=(m == 0),
                stop=(m == 7),
            )

        # ---- local softmax stats + partial context ----
        m_l = pc.tile([1, 1], F32, name="m_l")
        nc.vector.reduce_max(m_l[:], score_ps[:], axis=AX.X)
        neg_ml = pc.tile([1, 1], F32, name="neg_ml")
        nc.vector.reduce_max(neg_ml[:], score_ps[:], axis=AX.X, negate=True)
        e_row = pc.tile([1, LS], F32, name="e_row")
        S_l = pc.tile([1, 1], F32, name="S_l")
        nc.scalar.activation(
            e_row[:], score_ps[:], AF.Exp, bias=neg_ml[:], accum_out=S_l[:]
        )
        # e as columns for the context matmul
        ecol = pc.tile([128, 2], F32, name="ecol")
        for j in range(2):
            ps = ps_t.tile([128, 1], F32, name="ec_ps", tag="psT")
            nc.tensor.matmul(
                ps[:],
                e_row[0:1, j * 128 : (j + 1) * 128],
                ones_sb[:],
                start=True,
                stop=True,
            )
            nc.vector.tensor_copy(ecol[:, j : j + 1], ps[:])

        stats = pc.tile([1, 2056], F32, name="stats")
        nc.vector.tensor_copy(stats[0:1, 0:1], m_l[:])
        nc.vector.tensor_copy(stats[0:1, 1:2], S_l[:])
        nc.vector.memset(stats[0:1, 2:8], 0.0)
        for n in range(4):
            ps = ps_r.tile([1, 512], F32, name="ctxp_ps", tag="psR")
            for j in range(2):
                nc.tensor.matmul(
                    ps[:],
                    ecol[:, j : j + 1],
                    enc_sb[:, j * 2 * H + n * 512 : j * 2 * H + (n + 1) * 512],
                    start=(j == 0),
                    stop=(j == 1),
                )
            nc.vector.tensor_copy(stats[0:1, 8 + n * 512 : 8 + (n + 1) * 512], ps[:])

        # ---- AllGather #1: (max, sumexp, partial context) ----
        cc1_in = pdram.tile([1, 2056], F32, name="cc1_in")
        cc1_out = pdram.tile([8, 2056], F32, name="cc1_out", addr_space="Local" if no_cc else "Shared")
        nc.scalar.dma_start(cc1_in[:], stats[:])
        if no_cc:
            for _r in range(NCORES):
                nc.scalar.dma_start(cc1_out[_r : _r + 1, :], cc1_in[:])
        else:
            nc.gpsimd.collective_compute(
                "AllGather",
                ALU.bypass,
                replica_groups=RG,
                ins=[cc1_in.opt()],
                outs=[cc1_out.opt()],
            )

        # ---- output layer phase 1 (emb|hidden rows) — runs while AG1 is in
        # flight; emitted here so PE's in-order stream isn't blocked behind
        # anything that depends on the collective.
        # partial holds phase-1 sums per 512-slot, then (slots being
        # contiguous for every chunk but the ragged last) the final out row.
        partial = pc.tile([1, npart], F32, name="partial")
        out_chs = [] if no_out else chs
        if no_out:
            nc.vector.memset(partial[:], 0.0)

        # phase 1: K rows 0:2048 ([emb | hidden]) — no ctx dependency
        for ci, (c0, cn) in enumerate(out_chs):
            w1 = pw_out.tile([128, 16 * 512], F32R, name="w1", tag="wout")
            nc.sync.dma_start(
                w1[:, 0 : 16 * cn],
                wout_d[0 : 2 * H, c0 : c0 + cn].rearrange(
                    "(k p) c -> p k c", p=128
                ),
            )
            ps = ps_r.tile([1, 512], F32, name="po1", tag="psR")
            for k in range(16):
                lhs = emb_f[:, k : k + 1] if k < 8 else hidT[:, k - 8 : k - 7]
                nc.tensor.matmul(
                    ps[0:1, 0:cn],
                    lhs,
                    w1[:, k * cn : (k + 1) * cn],
                    start=(k == 0),
                    stop=(k == 15),
                )
            nc.vector.tensor_copy(
                partial[0:1, ci * 512 : ci * 512 + cn], ps[0:1, 0:cn]
            )

        crows = pc.tile([8, 2 * H], F32, name="crows")
        nc.scalar.dma_start(crows[:], cc1_out[0:8, 8:2056])
        ms1 = pc.tile([1, 8], F32, name="ms1")
        nc.scalar.dma_start(ms1[:], cc1_out.rearrange("r c -> c r")[0:1, 0:8])
        ss1 = pc.tile([1, 8], F32, name="ss1")
        nc.scalar.dma_start(ss1[:], cc1_out.rearrange("r c -> c r")[1:2, 0:8])

        # global combine (all on partition 0)
        negMg = pc.tile([1, 1], F32, name="negMg")
        nc.vector.reduce_max(negMg[:], ms1[:], axis=AX.X, negate=True)
        sc1 = pc.tile([1, 8], F32, name="sc1")
        nc.scalar.activation(sc1[:], ms1[:], AF.Exp, bias=negMg[:])
        t1 = pc.tile([1, 8], F32, name="t1")
        nc.vector.tensor_mul(t1[:], sc1[:], ss1[:])
        S_tot = pc.tile([1, 1], F32, name="S_tot")
        nc.vector.reduce_sum(S_tot[:], t1[:], axis=AX.X)
        rinv = pc.tile([1, 1], F32, name="rinv")
        nc.vector.reciprocal(rinv[:], S_tot[:])
        sc2 = pc.tile([1, 8], F32, name="sc2")
        nc.vector.tensor_scalar_mul(sc2[:], sc1[:], rinv[:])

        # attn_weights output for this core's L-shard
        aw_s = pc.tile([1, 1], F32, name="aw_s")
        nc.scalar.activation(aw_s[:], m_l[:], AF.Exp, bias=negMg[:])
        aw_s2 = pc.tile([1, 1], F32, name="aw_s2")
        nc.vector.tensor_mul(aw_s2[:], aw_s[:], rinv[:])
        aw_row = pc.tile([1, LS], F32, name="aw_row")
        nc.vector.tensor_scalar_mul(aw_row[:], e_row[:], aw_s2[:])
        nc.scalar.dma_start(o_aw[:, :], aw_row[:])

        # context = sc2 @ crows  -> [1, 2048]
        sc2c_ps = ps_t.tile([8, 1], F32, name="sc2c_ps", tag="psT")
        nc.tensor.matmul(sc2c_ps[:], sc2[:], ones_sb[:], start=True, stop=True)
        sc2col = pc.tile([8, 1], F32, name="sc2col")
        nc.vector.tensor_copy(sc2col[:], sc2c_ps[:])
        ctx_row = pc.tile([1, 2 * H], F32, name="ctx_row")
        for n in range(4):
            ps = ps_r.tile([1, 512], F32, name="ctx_ps", tag="psR")
            nc.tensor.matmul(
                ps[:],
                sc2col[:],
                crows[:, n * 512 : (n + 1) * 512],
                start=True,
                stop=True,
            )
            nc.vector.tensor_copy(ctx_row[0:1, n * 512 : (n + 1) * 512], ps[:])

        # unrounded fp32 copies of [emb | ctx | hidden] columns for the LSTM
        # (keeps the gates at full fp32 precision; fp32r wouldn't speed up
        # N=128 matmuls anyway)
        xT32 = pc.tile([128, 32], F32, name="xT32")
        nc.vector.tensor_add(xT32[:, 0:8], vecs_sb[:, 0:8], vecs_sb[:, 8:16])
        nc.vector.tensor_copy(xT32[:, 24:32], vecs_sb[:, 16:24])

        # context transposed columns ctxT [128, 16]
        ctxT = pc.tile([128, 16], F32R, name="ctxT")
        for t in range(16):
            ps = ps_t.tile([128, 1], F32, name="ctxT_ps", tag="psT")
            nc.tensor.matmul(
                ps[:],
                ctx_row[0:1, t * 128 : (t + 1) * 128],
                ones_sb[:],
                start=True,
                stop=True,
            )
            nc.vector.tensor_copy(ctxT[:, t : t + 1], ps[:])
            nc.vector.tensor_copy(xT32[:, 8 + t : 9 + t], ps[:])

        def xT_lstm(k):
            # LSTM input order: [emb, context, hidden]
            return xT32[:, k : k + 1]

        # ---- LSTM gates (this core's 128 output columns) ----
        gates = []
        for g in range(4):
            psg = ps_r.tile([1, 128], F32, name="psg", tag="psR")
            for half in range(2):
                wg = pw_misc.tile([128, 16 * 128], F32, name="wg", tag="wmisc")
                nc.scalar.dma_start(
                    wg[:],
                    wg_d[g][half * 2 * H : (half + 1) * 2 * H, :]
                    .rearrange("(k p) c -> p k c", p=128),
                )
                for kk in range(16):
                    k = half * 16 + kk
                    nc.tensor.matmul(
                        psg[:],
                        xT_lstm(k),
                        wg[:, kk * 128 : (kk + 1) * 128],
                        start=(k == 0),
                        stop=(k == 31),
                    )
            gsum = pc.tile([1, 128], F32, name=f"gsum{g}", tag=f"gsum{g}")
            nc.vector.tensor_add(
                gsum[:], psg[:], rows_sb[0:1, 1024 + g * 128 : 1024 + (g + 1) * 128]
            )
            gact = pc.tile([1, 128], F32, name=f"gact{g}", tag=f"gact{g}")
            nc.scalar.activation(
                gact[:], gsum[:], AF.Tanh if g == 3 else AF.Sigmoid
            )
            gates.append(gact)
        i_g, o_g, f_g, c_g = gates

        fc = pc.tile([1, 128], F32, name="fc")
        nc.vector.tensor_mul(fc[:], f_g[:], rows_sb[0:1, 1536:1664])
        ic = pc.tile([1, 128], F32, name="ic")
        nc.vector.tensor_mul(ic[:], i_g[:], c_g[:])
        cnew = pc.tile([1, 128], F32, name="cnew")
        nc.vector.tensor_add(cnew[:], fc[:], ic[:])
        nc.scalar.dma_start(o_c[:, :], cnew[:])
        ctanh = pc.tile([1, 128], F32, name="ctanh")
        nc.scalar.activation(ctanh[:], cnew[:], AF.Tanh)
        hnew = pc.tile([1, 128], F32, name="hnew")
        nc.vector.tensor_mul(hnew[:], o_g[:], ctanh[:])
        nc.scalar.dma_start(o_h[:, :], hnew[:])

        # ---- output layer: out = [emb|hidden|ctx] @ W_out + b_out ----

        # phase 2: K rows 2048:4096 (context) + finalize
        for ci, (c0, cn) in enumerate(out_chs):
            w2 = pw_out.tile([128, 16 * 512], F32R, name="w2", tag="wout")
            nc.sync.dma_start(
                w2[:, 0 : 16 * cn],
                wout_d[2 * H : 4 * H, c0 : c0 + cn].rearrange(
                    "(k p) c -> p k c", p=128
                ),
            )
            ps = ps_r.tile([1, 512], F32, name="po2", tag="psR")
            for k in range(16):
                nc.tensor.matmul(
                    ps[0:1, 0:cn],
                    ctxT[:, k : k + 1],
                    w2[:, k * cn : (k + 1) * cn],
                    start=(k == 0),
                    stop=(k == 15),
                )
            bout_c = pc.tile([1, 512], F32, name="bout_c", tag="boutc", bufs=2)
            nc.scalar.dma_start(bout_c[0:1, 0:cn], bout_d[0:1, c0 : c0 + cn])
            nc.vector.tensor_add(
                ps[0:1, 0:cn], ps[0:1, 0:cn], partial[0:1, ci * 512 : ci * 512 + cn]
            )
            nc.vector.tensor_add(
                partial[0:1, c0 : c0 + cn], ps[0:1, 0:cn], bout_c[0:1, 0:cn]
            )

        # ---- log-softmax (chunked local max / sum-exp over the out row) ----
        mx = pc.tile([1, 16], F32, name="mx")
        sx = pc.tile([1, 16], F32, name="sx")
        for ci, (c0, cn) in enumerate(chs):
            nc.vector.reduce_max(
                mx[0:1, ci : ci + 1], partial[0:1, c0 : c0 + cn], axis=AX.X
            )
        m2 = pc.tile([1, 1], F32, name="m2")
        nc.vector.reduce_max(m2[:], mx[0:1, 0 : len(chs)], axis=AX.X)
        neg_m2 = pc.tile([1, 1], F32, name="neg_m2")
        nc.vector.reduce_max(neg_m2[:], mx[0:1, 0 : len(chs)], axis=AX.X, negate=True)
        for ci, (c0, cn) in enumerate(chs):
            escr = pc.tile([1, 512], F32, name="escr", tag="escr", bufs=1)
            nc.scalar.activation(
                escr[0:1, 0:cn],
                partial[0:1, c0 : c0 + cn],
                AF.Exp,
                bias=neg_m2[:],
                accum_out=sx[0:1, ci : ci + 1],
            )
        S2 = pc.tile([1, 1], F32, name="S2")
        nc.vector.reduce_sum(S2[:], sx[0:1, 0 : len(chs)], axis=AX.X)
        st2 = pc.tile([1, 8], F32, name="st2")
        nc.vector.memset(st2[:], 0.0)
        nc.vector.tensor_copy(st2[0:1, 0:1], m2[:])
        nc.vector.tensor_copy(st2[0:1, 1:2], S2[:])

        cc2_in = pdram.tile([1, 8], F32, name="cc2_in")
        cc2_out = pdram.tile([8, 8], F32, name="cc2_out", addr_space="Local" if no_cc else "Shared")
        nc.scalar.dma_start(cc2_in[:], st2[:])
        if no_cc:
            for _r in range(NCORES):
                nc.scalar.dma_start(cc2_out[_r : _r + 1, :], cc2_in[:])
        else:
            nc.gpsimd.collective_compute(
                "AllGather",
                ALU.bypass,
                replica_groups=RG,
                ins=[cc2_in.opt()],
                outs=[cc2_out.opt()],
            )
        ms2 = pc.tile([1, 8], F32, name="ms2")
        nc.scalar.dma_start(ms2[:], cc2_out.rearrange("r c -> c r")[0:1, 0:8])
        ss2 = pc.tile([1, 8], F32, name="ss2")
        nc.scalar.dma_start(ss2[:], cc2_out.rearrange("r c -> c r")[1:2, 0:8])

        negMg2 = pc.tile([1, 1], F32, name="negMg2")
        nc.vector.reduce_max(negMg2[:], ms2[:], axis=AX.X, negate=True)
        e2 = pc.tile([1, 8], F32, name="e2")
        nc.scalar.activation(e2[:], ms2[:], AF.Exp, bias=negMg2[:])
        t2 = pc.tile([1, 8], F32, name="t2")
        nc.vector.tensor_mul(t2[:], e2[:], ss2[:])
        Sg2 = pc.tile([1, 1], F32, name="Sg2")
        nc.vector.reduce_sum(Sg2[:], t2[:], axis=AX.X)
        lnS = pc.tile([1, 1], F32, name="lnS")
        nc.scalar.activation(lnS[:], Sg2[:], AF.Ln)
        nlnS = pc.tile([1, 1], F32, name="nlnS")
        nc.vector.tensor_scalar_mul(nlnS[:], lnS[:], -1.0)
        nshift = pc.tile([1, 1], F32, name="nshift")
        nc.vector.tensor_add(nshift[:], negMg2[:], nlnS[:])

        nc.vector.tensor_scalar_add(
            partial[0:1, 0:vs], partial[0:1, 0:vs], nshift[:]
        )
        nc.scalar.dma_start(o_lsm[:, :], partial[0:1, 0:vs])

    nc.compile()
    return nc


def make_in_maps(inputs, vs=VS, vfull=V):
    """Shard the full inputs into 8 per-core input maps."""
    f32 = lambda x: np.ascontiguousarray(np.asarray(x), dtype=np.float32)

    tok = int(np.asarray(inputs["input_tok"]).reshape(-1)[0])
    W_emb = np.asarray(inputs["W_emb"])
    emb_row = f32(W_emb[tok])                       # [H]
    b_emb = f32(inputs["b_emb"])
    hidden = f32(inputs["hidden"]).reshape(H)
    cell = f32(inputs["cell_state"]).reshape(H)
    enc = f32(np.asarray(inputs["encoder_outputs"])[:, 0, :])   # [L, 2H]
    W_attn = f32(inputs["W_attn"])
    W_attn_out = f32(inputs["W_attn_out"]).reshape(H)
    W_out = np.asarray(inputs["W_out"])
    b_out = f32(inputs["b_out"]).reshape(-1)[:vfull]
    Wg_full = [f32(inputs[n]) for n in ("W_i", "W_o", "W_f", "W_c")]
    b_g = [f32(inputs[n]) for n in ("b_i", "b_o", "b_f", "b_c")]

    vecs = np.zeros((128, 32), np.float32)
    vecs[:, 0:8] = emb_row.reshape(8, 128).T
    vecs[:, 8:16] = b_emb.reshape(8, 128).T
    vecs[:, 16:24] = hidden.reshape(8, 128).T
    vecs[:, 24:32] = W_attn_out.reshape(8, 128).T
    ident = np.eye(128, dtype=np.float32)

    in_maps = []
    for r in range(NCORES):
        c0, c1 = r * vs, (r + 1) * vs
        if c1 <= vfull:
            w_sh = np.ascontiguousarray(W_out[:, c0:c1], dtype=np.float32)
            b_sh = b_out[c0:c1].reshape(1, vs).copy()
        else:
            w_sh = np.zeros((4 * H, vs), np.float32)
            b_sh = np.full((1, vs), NEG_BIG, np.float32)
            if c0 < vfull:
                w_sh[:, : vfull - c0] = W_out[:, c0:vfull]
                b_sh[0, : vfull - c0] = b_out[c0:vfull]
        rows = np.zeros((1, 1664), np.float32)
        rows[0, 0:1024] = f32(inputs["b_attn"])
        for g in range(4):
            rows[0, 1024 + g * 128 : 1024 + (g + 1) * 128] = b_g[g][
                r * 128 : (r + 1) * 128
            ]
        rows[0, 1536:1664] = cell[r * 128 : (r + 1) * 128]
        wg = np.stack(
            [w[:, r * 128 : (r + 1) * 128] for w in Wg_full], axis=0
        ).astype(np.float32)
        in_maps.append(
            {
                "enc": np.ascontiguousarray(enc[r * LS : (r + 1) * LS]),
                "w_attn": W_attn,
                "w_out": w_sh,
                "b_out": b_sh,
                "w_g": np.ascontiguousarray(wg),
                "vecs": vecs,
                "rows": rows,
                "ident": ident,
            }
        )
    return in_maps


def assemble_outputs(results, vs=VS, vfull=V):
    lsm = np.concatenate([results[r]["o_lsm"][0] for r in range(NCORES)])[:vfull]
    h = np.concatenate([results[r]["o_h"][0] for r in range(NCORES)])
    c = np.concatenate([results[r]["o_c"][0] for r in range(NCORES)])
    aw = np.concatenate([results[r]["o_aw"][0] for r in range(NCORES)])
    return (
        lsm.reshape(1, vfull).astype(np.float32),
        h.reshape(1, H).astype(np.float32),
        aw.reshape(L, 1).astype(np.float32),
        c.reshape(1, H).astype(np.float32),
    )


_PROGRAM = None


def _program():
    global _PROGRAM
    if _PROGRAM is None:
        _PROGRAM = build_program()
    return _PROGRAM


def kernel(**inputs):
    nc = _program()
    in_maps = make_in_maps(inputs)
    res = run_bass_kernel_spmd(nc, in_maps, core_ids=list(range(NCORES)))
    return assemble_outputs(res.results)
